# revision 15
# baseline (speedup 1.0000x reference)
"""MoE (4 MLP experts + 4 FasterKAN experts, top-2) Trainium2 kernel.

Sharding: expert-parallel, feature-split across core pairs. The router (tiny)
runs on the host as part of input sharding. Cores (2p, 2p+1) own MLP expert p
and KAN expert 4+p; each core processes ALL of its experts' routed tokens
(capacity 512; overflow handled exactly on host) but only HALF of each
expert's feature dimension:

  MLP:  core side s computes h = gelu(x @ W1[:, sF:sF+F/2]) and the partial
        y_s = h @ W2[sF:sF+F/2, :]; host sums y_0 + y_1. No cross-core traffic.
  KAN:  core side s computes z-half = basis0 @ W0[:, half] (layer-0 output
        features split) and layer-1 partial y_s from its own z-half's basis
        rows (K split); host sums partials. LayerNorm-1 needs mean/var over
        the FULL F2 features -> tiny [1,1024] fp32 AllReduce over the core
        pair (Σz | Σz² packed in one row), overlapped with the MLP phase.

Device numerics: all matmuls bf16 with fp32 PSUM accumulation, N=512 free
dim, kt-outer accumulation with ring-buffered RSWAF basis tiles (never fully
resident). LN column stats via PE ones-matmul; rstd via DVE reciprocal seed
+ Newton iterations. The G=8 RSWAF grid functions are compressed to NA=3
fitted sech^2 atoms + a constant (grid dim pre-mixed into the spline weights
on the host), shrinking the KAN K dim from G*H to NA*H; the constant term
and the "+1" of the -tanh^2 form fold into the output bias.
"""

import os

import numpy as np
import ml_dtypes

import concourse.bass as bass
import concourse.tile as tile
from concourse import bacc, mybir
from concourse import bass_utils

BF16 = ml_dtypes.bfloat16

# ---- problem constants (hardcoded per contract) ----
T, H, F, E = 2048, 1024, 4096, 8
F2 = F // 2
E2 = E // 2
G = 8
TOP_K = 2
INV_DENOM = 0.5
GRID = np.linspace(-1.2, 0.2, G).astype(np.float32)
LN_EPS = 1e-5
P = 128
C = 512            # capacity per expert (all tokens; overflow -> host)
HT = H // P        # 8 H-tiles
FH = F // 2        # 2048: MLP F half per core
FHT = FH // P      # 16
F2H = F2 // 2      # 1024: KAN z-feature half per core
F2HT = F2H // P    # 8

# ---- low-rank RSWAF basis compression ----
# The 8 grid basis functions sech^2((x-c_g)/2), c_g in linspace(-1.2,0.2,8),
# are approximated (N(0,1)-weighted LSQ, the post-LN input distribution) by
#   b_g(x) ~= COEF[0,g] + sum_r COEF[1+r,g] * sech^2(ATOM_A[r]*x + ATOM_B[r])
# with NA=3 fitted atoms (weighted RMS 1.2e-3; end-to-end rel err 1.6e-3).
# The grid dim is mixed into the spline weights on the host, so the device
# K dim shrinks from G*H to NA*H (2.67x less PE work for the KAN experts).
NA = 3
ATOM_A = np.array([0.5104, 0.4978, 0.4977], np.float64)
ATOM_B = np.array([0.2581, 0.5374, -0.0356], np.float64)
COEF = np.array([
    [-7.49781605e-03,  3.85725206e-03,  8.71658627e-03,  6.63497338e-03,
     -2.92025891e-04, -7.25583852e-03, -7.22697257e-03,  7.91052327e-03],
    [-3.26024659e-01,  2.25285923e-01,  6.30653143e-01,  8.36926569e-01,
      8.18689331e-01,  5.85328647e-01,  1.81687157e-01, -3.18349195e-01],
    [ 1.25153196e+00,  8.26446110e-01,  4.54749619e-01,  1.71840623e-01,
     -1.45976379e-03, -6.35530915e-02, -3.30161903e-02,  5.46003779e-02],
    [ 7.34034472e-02, -5.47935817e-02, -8.74939226e-02, -4.93450897e-03,
      1.95608070e-01,  4.96872627e-01,  8.62470677e-01,  1.24173739e+00],
], np.float64)

KT = NA * 8        # 24 K-tiles for both KAN layers per core (3*1024 / 128)

last_run_info = {}


def _register_ntff_hook():
    """Best-effort NTFF profiling hook registration (used when BASS_TRACE=1)."""
    try:
        import sys
        import types
        try:
            from antenv import axon_hooks  # noqa: F401
        except ImportError:
            # the image's antenv lacks axon_hooks; install a functional shim
            # so bass_utils' `from antenv.axon_hooks import ...` resolves
            import antenv
            mod = types.ModuleType("antenv.axon_hooks")
            mod._hook = None
            mod.set_axon_ntff_profile_hook = \
                lambda h, _m=mod: setattr(_m, "_hook", h)
            mod.get_axon_ntff_profile_hook = lambda _m=mod: _m._hook
            antenv.axon_hooks = mod
            sys.modules["antenv.axon_hooks"] = mod
        from antenv.axon_hooks import set_axon_ntff_profile_hook, \
            get_axon_ntff_profile_hook
        if get_axon_ntff_profile_hook() is not None:
            return
        from trn_agent_boot.trn_boot import _ntff_profile_via_ctypes
        so = "/opt/axon/libaxon_pjrt.so"
        if os.path.exists(so):
            set_axon_ntff_profile_hook(_ntff_profile_via_ctypes(so))
            # artifact upload needs a cloud bucket; keep artifacts local
            bass_utils.upload_artifacts = lambda tmpdir: tmpdir
    except Exception:
        pass


# --------------------------------------------------------------------------
# host-side routing (the dispatch half of the sharding strategy)
# --------------------------------------------------------------------------

def _route(x, gate_w):
    """Replicates the reference router in fp32. Returns (sel, w_full)."""
    logits = x.astype(np.float32) @ gate_w.astype(np.float32)        # [T, E]
    m = logits.max(axis=-1, keepdims=True)
    p = np.exp(logits - m, dtype=np.float32)
    probs = p / p.sum(axis=-1, keepdims=True, dtype=np.float32)
    # jax.lax.top_k semantics: descending, ties -> lower index first
    sel = np.argsort(-probs, axis=-1, kind="stable")[:, :TOP_K]      # [T, K]
    rw = np.take_along_axis(probs, sel, axis=-1)
    rw = rw / rw.sum(axis=-1, keepdims=True)
    w_full = np.zeros((T, E), np.float32)
    np.put_along_axis(w_full, sel, rw.astype(np.float32), axis=-1)
    return sel, w_full


# --------------------------------------------------------------------------
# host-side weight pre-tiling
# --------------------------------------------------------------------------

def _pretile_grouped(w, n_kt, n_mt, group):
    """[K, M] fp32 -> [n_mt/group, P, group*n_kt*P] bf16:
    out[gi, kp, ml*n_kt*P + kt*P + m] = w[kt*P+kp, (gi*group+ml)*P+m]."""
    a = w.reshape(n_kt, P, n_mt, P).transpose(2, 1, 0, 3)    # [mt, kp, kt, m]
    a = a.reshape(n_mt // group, group, P, n_kt, P).transpose(0, 2, 1, 3, 4)
    return np.ascontiguousarray(
        a.reshape(n_mt // group, P, group * n_kt * P).astype(BF16))


def _pretile_ktmajor(w, n_kt, n_mt, group):
    """kt-major: out[ci, kp, ktl*n_mt*P + mt*P + m] = w[(ci*group+ktl)*P+kp,
    mt*P+m] — one chunk holds `group` consecutive K-tiles across all mt."""
    a = w.reshape(n_kt // group, group, P, n_mt * P)         # [ci, ktl, kp, M]
    a = a.transpose(0, 2, 1, 3)
    return np.ascontiguousarray(
        a.reshape(n_kt // group, P, group * n_mt * P).astype(BF16))


def _pack_pp(v):
    """[n*P] fp32 per-feature vector -> [P, n] (partition-major) fp32."""
    n = v.shape[0] // P
    return np.ascontiguousarray(v.reshape(n, P).T.astype(np.float32))


def _prep_side_mlp(w1, b1, w2, b2, s):
    """Feature-half s of one MLP expert."""
    lo, hi = s * FH, (s + 1) * FH
    w1h = w1[:, lo:hi]                               # [H, FH]
    w2h = w2[lo:hi, :]                               # [FH, H]
    b2e = b2 if s == 0 else np.zeros_like(b2)
    return {
        "w1": _pretile_grouped(w1h, HT, FHT, 4),     # [4, 128, 4096]
        "w2": _pretile_grouped(w2h, FHT, HT, 2),     # [4, 128, 4096]
        "b1": _pack_pp(b1[lo:hi]),                   # [128, 16]
        "b2": _pack_pp(b2e),                         # [128, 8]
    }


def _mix_kan_expert(w0, w1):
    """Mix the grid dim of one KAN expert's spline weights with the fitted
    atom coefficients (once per expert; both sides slice the result).

    Returns (w0r [NA*H, F2], c0 [F2], w1g [F2, G, H], w1r [NA*F2, H]) where
    w0r/w1r rows are atom-major (atom r block, then feature) and c0 is the
    layer-0 constant-term fold COEF[0] applied to the grid dim."""
    cf = COEF.astype(np.float32)
    w0g = w0.reshape(H, G, F2)                       # rows (h, g)
    w0r = np.einsum('rg,hgf->rhf', cf[1:], w0g).reshape(NA * H, F2)
    c0 = COEF[0] @ w0g.sum(0, dtype=np.float64)      # [F2]
    w1g = w1.reshape(F2, G, H)
    w1r = np.einsum('rg,fgh->rfh', cf[1:], w1g).reshape(NA * F2, H)
    return w0r, c0, w1g, w1r


def _prep_side_kan(g0, b0, sb0, g1, b1, sb1, mixed, s):
    """Feature-half s of one KAN expert (z features / layer-1 K rows)."""
    w0r, c0, w1g, w1r = mixed
    lo, hi = s * F2H, (s + 1) * F2H
    w0h = w0r[:, lo:hi]                              # [3H, F2H]
    # layer 1: atom-major rows for OWN z-half features
    w1h = w1r.reshape(NA, F2, H)[:, lo:hi].reshape(NA * F2H, H)
    # bias fold: spline bias + const-term fold + "+1" of the -tanh^2 form
    bias0 = (sb0[lo:hi].astype(np.float64) + c0[lo:hi]
             + w0h.astype(np.float64).sum(0)).astype(np.float32)
    sb1e = sb1 if s == 0 else np.zeros_like(sb1)
    bias1 = (sb1e.astype(np.float64)
             + COEF[0] @ w1g[lo:hi].sum(0, dtype=np.float64)
             + w1h.astype(np.float64).sum(0)).astype(np.float32)
    # tanh scale/bias tables, col kt = r*8 + ft:
    #   tanh(a_r*(gamma*u + beta) + b_r) -> scale = a_r*gamma, bias = a_r*beta + b_r
    g0p, b0p = _pack_pp(g0), _pack_pp(b0)            # [128, 8]
    g1p, b1p = _pack_pp(g1[lo:hi]), _pack_pp(b1[lo:hi])
    sc0 = np.concatenate([ATOM_A[r] * g0p for r in range(NA)], 1)
    gb0 = np.concatenate([ATOM_A[r] * b0p + ATOM_B[r] for r in range(NA)], 1)
    sc1 = np.concatenate([ATOM_A[r] * g1p for r in range(NA)], 1)
    gb1 = np.concatenate([ATOM_A[r] * b1p + ATOM_B[r] for r in range(NA)], 1)
    return {
        "w0": _pretile_ktmajor(w0h, KT, F2HT, 8),    # [3, 128, 8192]
        "w1k": _pretile_ktmajor(w1h, KT, HT, 8),     # [3, 128, 8192]
        "bias0": _pack_pp(bias0),                    # [128, 8]
        "bias1": _pack_pp(bias1),                    # [128, 8]
        "sc0": np.ascontiguousarray(sc0.astype(np.float32)),   # [128, 24]
        "gb0": np.ascontiguousarray(gb0.astype(np.float32)),
        "sc1": np.ascontiguousarray(sc1.astype(np.float32)),
        "gb1": np.ascontiguousarray(gb1.astype(np.float32)),
    }


# --------------------------------------------------------------------------
# device program
# --------------------------------------------------------------------------

def _emit_stat_ft(nc, pools, x_sb, ft, n_ft, psx, psx2, ones_sb,
                  square_on_act=False):
    """One feature tile's contribution to column mean / mean-square.

    ones_sb carries 1/D so PSUM accumulates E[x] and E[x^2] directly.
    square_on_act routes the elementwise square to the scalar engine
    (Square shares every ACT table) when the DVE is the busier engine."""
    sbuf = pools["sbuf"]
    x2 = sbuf.tile([P, C], mybir.dt.bfloat16, tag="x2")
    if square_on_act:
        nc.scalar.activation(x2[:], x_sb[:, ft, :],
                             mybir.ActivationFunctionType.Square)
    else:
        nc.vector.tensor_tensor(x2[:], x_sb[:, ft, :], x_sb[:, ft, :],
                                op=mybir.AluOpType.mult)
    nc.tensor.matmul(psx[:], ones_sb[:], x_sb[:, ft, :],
                     start=(ft == 0), stop=(ft == n_ft - 1))
    nc.tensor.matmul(psx2[:], ones_sb[:], x2[:],
                     start=(ft == 0), stop=(ft == n_ft - 1))


def _emit_ln_rows(nc, pools, mean_ap, ex2_ap):
    """Row math: (E[x], E[x^2]) -> (rstd, -mu*rstd) as bf16 [1, C] rows.

    rstd = rsqrt(var + eps) entirely on DVE: quake-style magic seed
    (0x5f3759df) + one Newton iteration (rel err ~2e-3, far inside the
    bf16 downstream precision). Avoids ACT Sqrt so the scalar engine
    never swaps activation tables mid-kernel.
    """
    rows = pools["rows"]
    f32, u32 = mybir.dt.float32, mybir.dt.uint32
    var = rows.tile([1, C], f32, tag="row")
    t = rows.tile([1, C], f32, tag="row")
    r0 = rows.tile([1, C], u32, tag="row")
    rstd = rows.tile([1, C], mybir.dt.bfloat16, tag="rowb")
    negmr = rows.tile([1, C], mybir.dt.bfloat16, tag="rowb")
    if mean_ap.space == bass.MemorySpace.PSUM:
        # ops may read at most one non-scalar PSUM input
        mcopy = rows.tile([1, C], f32, tag="row")
        nc.vector.tensor_scalar_mul(mcopy[:], mean_ap, 1.0)
        mean_ap = mcopy[:]
    nc.vector.scalar_tensor_tensor(t[:], mean_ap, -1.0, mean_ap,
                                   op0=mybir.AluOpType.mult,
                                   op1=mybir.AluOpType.mult)     # -mean^2
    nc.vector.scalar_tensor_tensor(var[:], t[:], LN_EPS, ex2_ap,
                                   op0=mybir.AluOpType.add,
                                   op1=mybir.AluOpType.add)      # var + eps
    nc.vector.tensor_scalar(r0[:], var[:].bitcast(u32), 1, None,
                            op0=mybir.AluOpType.logical_shift_right)
    # magic - s without u32 wraparound (DVE arith may run via fp32)
    nc.vector.scalar_tensor_tensor(r0[:], pools["magic"][:].bitcast(u32),
                                   1.0, r0[:],
                                   op0=mybir.AluOpType.mult,
                                   op1=mybir.AluOpType.subtract)
    rf = r0[:].bitcast(f32)
    nc.vector.tensor_tensor(t[:], rf, rf, op=mybir.AluOpType.mult)
    nc.vector.tensor_tensor(t[:], t[:], var[:], op=mybir.AluOpType.mult)
    nc.vector.tensor_scalar(t[:], t[:], -0.5, 1.5,
                            op0=mybir.AluOpType.mult,
                            op1=mybir.AluOpType.add)             # 1.5-.5vr^2
    nc.vector.tensor_tensor(rstd[:], rf, t[:], op=mybir.AluOpType.mult)
    nc.vector.scalar_tensor_tensor(negmr[:], mean_ap, -1.0, rstd[:],
                                   op0=mybir.AluOpType.mult,
                                   op1=mybir.AluOpType.mult)     # -mu*rstd
    return rstd, negmr


def _emit_ln_bcast(nc, pools, psum_bc, rstd, negmr):
    """Per-layer [P, C] broadcasts of rstd and -mu*rstd (PE rank-1 outer with
    a bf16 ones row, drained to bf16 SBUF)."""
    bvec = pools["bvec"]
    onesf = pools["onesf"]
    br_ps = psum_bc.tile([P, C], mybir.dt.float32, tag="bcast")
    bm_ps = psum_bc.tile([P, C], mybir.dt.float32, tag="bcast")
    nc.tensor.matmul(br_ps[:], onesf[:], rstd[:], start=True, stop=True)
    nc.tensor.matmul(bm_ps[:], onesf[:], negmr[:], start=True, stop=True)
    br = bvec.tile([P, C], mybir.dt.bfloat16, tag="bvec")
    bm = bvec.tile([P, C], mybir.dt.bfloat16, tag="bvec")
    nc.scalar.activation(br[:], br_ps[:], mybir.ActivationFunctionType.Identity)
    nc.scalar.activation(bm[:], bm_ps[:], mybir.ActivationFunctionType.Identity)
    return br, bm


def _emit_u_ft(nc, pools, x_sb, ft, br, bm, u_tag, bufs, name=None):
    """u = x * br + bm (the LN affine transform is folded into the tanh)."""
    sbuf = pools["sbuf"]
    u = sbuf.tile([P, C], mybir.dt.bfloat16, tag=u_tag, bufs=bufs,
                  name=name or "u")
    nc.vector.tensor_tensor(u[:], x_sb[:, ft, :], br[:],
                            op=mybir.AluOpType.mult)
    nc.vector.tensor_tensor(u[:], u[:], bm[:], op=mybir.AluOpType.add)
    return u


def _emit_basis(nc, pools, u, scale_ap, bias_ap, tag="bas"):
    """ring tile = -tanh^2(u*(gamma/2) + (beta/2 - grid[g]/2))."""
    sbuf = pools["sbuf"]
    th = sbuf.tile([P, C], mybir.dt.bfloat16, tag="th", bufs=3)
    nc.scalar.activation(th[:], u[:],
                         mybir.ActivationFunctionType.Tanh,
                         bias=bias_ap, scale=scale_ap)
    b = sbuf.tile([P, C], mybir.dt.bfloat16, tag=tag, bufs=9)
    nc.vector.scalar_tensor_tensor(b[:], th[:], -1.0, th[:],
                                   op0=mybir.AluOpType.mult,
                                   op1=mybir.AluOpType.mult)
    return b


def _build_program():
    nc = bacc.Bacc("TRN2", target_bir_lowering=False, debug=False,
                   num_devices=8)
    dt_bf = mybir.dt.bfloat16
    dt_f32 = mybir.dt.float32

    d = {}
    d["xm"] = nc.dram_tensor("xm", [P, HT, C], dt_bf, kind="ExternalInput")
    d["xk"] = nc.dram_tensor("xk", [P, HT, C], dt_bf, kind="ExternalInput")
    d["w1"] = nc.dram_tensor("w1", [FHT // 4, P, 4 * HT * P], dt_bf,
                             kind="ExternalInput")
    d["w2"] = nc.dram_tensor("w2", [HT // 2, P, 2 * FHT * P], dt_bf,
                             kind="ExternalInput")
    d["w0"] = nc.dram_tensor("w0", [KT // 8, P, 8 * F2HT * P], dt_bf,
                             kind="ExternalInput")
    d["w1k"] = nc.dram_tensor("w1k", [KT // 8, P, 8 * HT * P], dt_bf,
                              kind="ExternalInput")
    # packed consts [P, 136]: b1(16) b2(8) bias0(8) bias1(8)
    #   sc0(24) gb0(24) sc1(24) gb1(24)  (tanh scale/bias, col = r*8+ft)
    d["cst"] = nc.dram_tensor("cst", [P, 136], dt_f32, kind="ExternalInput")
    d["ym"] = nc.dram_tensor("ym", [H, C], dt_f32, kind="ExternalOutput")
    d["yk"] = nc.dram_tensor("yk", [H, C], dt_f32, kind="ExternalOutput")

    with tile.TileContext(nc) as tc:
        with (
            tc.tile_pool(name="const", bufs=1) as const,
            tc.tile_pool(name="acts", bufs=1) as acts,
            tc.tile_pool(name="work", bufs=3) as work,
            tc.tile_pool(name="bvecp", bufs=4) as bvecp,
            tc.tile_pool(name="wstream", bufs=4) as wstream,
            tc.tile_pool(name="rows", bufs=5) as rows,
            tc.tile_pool(name="ystage", bufs=3) as ystage,
            tc.tile_pool(name="dram", bufs=1, space="DRAM") as dram,
        ):
            # ---- input/const DMAs (xk first: LN0 gates the KAN pipeline) ----
            xk_sb = acts.tile([P, HT, C], dt_bf)
            nc.sync.dma_start(xk_sb[:], d["xk"].ap())
            xm_sb = acts.tile([P, HT, C], dt_bf)
            nc.sync.dma_start(xm_sb[:], d["xm"].ap())
            cst_sb = const.tile([P, 136], dt_f32)
            nc.gpsimd.dma_start(cst_sb[:], d["cst"].ap())
            b1_sb = cst_sb[:, 0:16]
            b2_sb = cst_sb[:, 16:24]
            bias0_sb = cst_sb[:, 24:32]
            bias1_sb = cst_sb[:, 32:40]
            SC0, GB0, SC1, GB1 = 40, 64, 88, 112

            ones0_sb = const.tile([P, 1], dt_bf)     # 1/H for LN0 stats
            nc.vector.memset(ones0_sb[:], 1.0 / H)
            ones1_sb = const.tile([P, 1], dt_bf)     # 1/F2 for LN1 stats
            nc.vector.memset(ones1_sb[:], 1.0 / F2)
            onesf_sb = const.tile([1, P], dt_bf)
            nc.vector.memset(onesf_sb[:], 1.0)
            # fp32 value whose bit pattern is the rsqrt magic 0x5F3759DF
            magic_sb = const.tile([1, C], dt_f32)
            nc.vector.memset(magic_sb[:], 1.3211836172961055e+19)

            pools = {"sbuf": work, "rows": rows,
                     "onesf": onesf_sb, "bvec": bvecp, "magic": magic_sb}

            # table preloads: dummy Gelu+Tanh force the ACT table load(s)
            # during the input-DMA dead time instead of mid-pipeline
            tw_sb = const.tile([1, 2], dt_bf)
            nc.scalar.activation(tw_sb[:, 0:1], magic_sb[0:1, 0:1],
                                 mybir.ActivationFunctionType.Gelu)
            nc.scalar.activation(tw_sb[:, 1:2], magic_sb[0:1, 0:1],
                                 mybir.ActivationFunctionType.Tanh)
            # PE warm burst: ~4us of tiny matmuls un-throttle the HAM clock
            # gate before the first real matmuls arrive
            wz_sb = const.tile([P, 64], dt_bf)
            nc.vector.memset(wz_sb[:], 0.0)

            h_sb = acts.tile([P, FHT, C], dt_bf)     # MLP hidden (GELU'd)
            z_sb = acts.tile([P, F2HT, C], dt_bf)    # KAN z half

            # stats packed row for the pair AllReduce: [Σz | Σz²]
            sums_sb = rows.tile([1, 2 * C], dt_f32, name="sums")
            sums2_sb = rows.tile([1, 2 * C], dt_f32, name="sums2")
            cc_in = dram.tile([1, 2 * C], dt_f32)
            cc_out = dram.tile([1, 2 * C], dt_f32)

            # warmup collective: absorbs the ncfw control-plane startup cost
            # (~40-75us) so the real stats AllReduce later completes fast.
            # Input is an uninitialized internal scratch tile (summed garbage
            # is never read) so the doorbell carries no DMA dependency and
            # never blocks the gpsimd queue.
            ccw_in = dram.tile([1, P], dt_f32)
            ccw_out = dram.tile([1, P], dt_f32)
            nc.gpsimd.collective_compute(
                "AllReduce", mybir.AluOpType.add,
                replica_groups=[[0, 1], [2, 3], [4, 5], [6, 7]],
                ins=[ccw_in.opt()], outs=[ccw_out.opt()])

            # ---- scope A: LN0 stats + bcast; MLP L1 first group ----
            with (
                tc.tile_pool(name="ps_stat0", bufs=2, space="PSUM") as ps_s0,
                tc.tile_pool(name="ps_bc0", bufs=2, space="PSUM") as ps_b0,
                tc.tile_pool(name="ps_mlp1", bufs=3, space="PSUM") as ps_m1,
                tc.tile_pool(name="ps_warm", bufs=1, space="PSUM") as ps_w,
            ):
                pswm = ps_w.tile([64, 64], dt_f32, tag="warm")
                for _ in range(64):
                    nc.tensor.matmul(pswm[:], wz_sb[:], wz_sb[:],
                                     start=True, stop=True)
                psx = ps_s0.tile([1, C], dt_f32, tag="stat")
                psx2 = ps_s0.tile([1, C], dt_f32, tag="stat")
                for ft in range(HT):
                    _emit_stat_ft(nc, pools, xk_sb, ft, HT, psx, psx2,
                                  ones0_sb, square_on_act=True)
                rstd0, negmr0 = _emit_ln_rows(nc, pools, psx[0:1, :],
                                              psx2[0:1, :])
                # bcast emitted BEFORE the MLP groups so the KAN L0 head
                # chain (bcast -> u -> tanh) completes behind the MLP work
                br0, bm0 = _emit_ln_bcast(nc, pools, ps_b0, rstd0, negmr0)

                # ---- MLP L1 groups 0-1 (fill the PE while LN0 ramps) ----
                def drain_h(mt, ps):
                    nc.scalar.activation(h_sb[:, mt, :], ps[:],
                                         mybir.ActivationFunctionType.Gelu,
                                         bias=b1_sb[:, mt:mt + 1], scale=1.0)

                def emit_mlp1_group(gi, pool):
                    wch = wstream.tile([P, 4 * HT * P], dt_bf, tag="wmlp",
                                       bufs=2)
                    nc.gpsimd.dma_start(wch[:], d["w1"].ap()[gi])
                    for ml in range(4):
                        mt = gi * 4 + ml
                        ps = pool.tile([P, C], dt_f32, tag="mm")
                        for kt in range(HT):
                            nc.tensor.matmul(
                                ps[:],
                                wch[:, (ml * HT + kt) * P:
                                    (ml * HT + kt + 1) * P],
                                xm_sb[:, kt, :],
                                start=(kt == 0), stop=(kt == HT - 1))
                        drain_h(mt, ps)

                emit_mlp1_group(0, ps_m1)
                emit_mlp1_group(1, ps_m1)

            # ---- scope B: KAN L0, kt-outer, 8 resident PSUM banks.  The
            # last weight chunk runs mt-inner so the banks complete (and
            # drain) staggered instead of all at once at kt==KT-1. ----
            u0 = {}
            with tc.tile_pool(name="ps_l0", bufs=F2HT, space="PSUM") as ps_l0:
                psz = [ps_l0.tile([P, C], dt_f32, tag="l0", name=f"psz{mt}")
                       for mt in range(F2HT)]

                def basis0_for(kt):
                    ft = kt % HT
                    if ft not in u0:
                        u0[ft] = _emit_u_ft(nc, pools, xk_sb, ft, br0,
                                            bm0, u_tag="u", bufs=HT,
                                            name=f"u0f{ft}")
                    return _emit_basis(nc, pools, u0[ft],
                                       cst_sb[:, SC0 + kt:SC0 + kt + 1],
                                       cst_sb[:, GB0 + kt:GB0 + kt + 1])

                def drain_z(mt):
                    if mt % 2 == 0:
                        nc.scalar.activation(
                            z_sb[:, mt, :], psz[mt][:],
                            mybir.ActivationFunctionType.Identity,
                            bias=bias0_sb[:, mt:mt + 1], scale=1.0)
                    else:
                        nc.vector.tensor_scalar_add(
                            z_sb[:, mt, :], psz[mt][:],
                            bias0_sb[:, mt:mt + 1])

                n_chunks = KT // 8
                for ci in range(n_chunks):
                    wch = wstream.tile([P, 8 * F2HT * P], dt_bf, tag="wkan",
                                       bufs=2)
                    nc.sync.dma_start(wch[:], d["w0"].ap()[ci])
                    if ci < n_chunks - 1:
                        for ktl in range(8):
                            kt = ci * 8 + ktl
                            b = basis0_for(kt)
                            for mt in range(F2HT):
                                nc.tensor.matmul(
                                    psz[mt][:],
                                    wch[:, (ktl * F2HT + mt) * P:
                                        (ktl * F2HT + mt + 1) * P],
                                    b[:],
                                    start=(kt == 0), stop=False)
                    else:
                        bs = [basis0_for(ci * 8 + ktl) for ktl in range(8)]
                        for mt in range(F2HT):
                            for ktl in range(8):
                                nc.tensor.matmul(
                                    psz[mt][:],
                                    wch[:, (ktl * F2HT + mt) * P:
                                        (ktl * F2HT + mt + 1) * P],
                                    bs[ktl][:],
                                    start=False, stop=(ktl == 7))
                            drain_z(mt)

            # ---- LN1 partial stats -> pack -> AllReduce over the pair ----
            with tc.tile_pool(name="ps_stat1", bufs=2, space="PSUM") as ps_s1:
                psx1 = ps_s1.tile([1, C], dt_f32, tag="stat")
                psx21 = ps_s1.tile([1, C], dt_f32, tag="stat")
                for mt in range(F2HT):
                    _emit_stat_ft(nc, pools, z_sb, mt, F2HT, psx1, psx21,
                                  ones1_sb)
                nc.vector.tensor_scalar_mul(sums_sb[:, 0:C], psx1[:], 1.0)
                nc.vector.tensor_scalar_mul(sums_sb[:, C:2 * C], psx21[:], 1.0)
            nc.gpsimd.dma_start(cc_in[:], sums_sb[:])
            nc.gpsimd.collective_compute(
                "AllReduce", mybir.AluOpType.add,
                replica_groups=[[0, 1], [2, 3], [4, 5], [6, 7]],
                ins=[cc_in.opt()], outs=[cc_out.opt()])

            # ---- scope C: MLP L1 g2-3, LN1 chain, MLP L2 (all 8 cols) ----
            with (
                tc.tile_pool(name="ps_mlp1b", bufs=2, space="PSUM") as ps_m1b,
                tc.tile_pool(name="ps_mlp2", bufs=2, space="PSUM") as ps_m2,
                tc.tile_pool(name="ps_bc1", bufs=2, space="PSUM") as ps_b1,
            ):
                emit_mlp1_group(2, ps_m1b)
                emit_mlp1_group(3, ps_m1b)

                # LN1 chain emitted BEFORE MLP L2 so its ACT/DVE ops are
                # prioritized ahead of the ym drains in the strict FIFOs
                nc.gpsimd.dma_start(sums2_sb[:], cc_out[:])
                rstd1, negmr1 = _emit_ln_rows(nc, pools,
                                              sums2_sb[0:1, 0:C],
                                              sums2_sb[0:1, C:2 * C])
                br1, bm1 = _emit_ln_bcast(nc, pools, ps_b1, rstd1, negmr1)

                def emit_mlp2_cols(gi, pool):
                    wch = wstream.tile([P, 2 * FHT * P], dt_bf, tag="wmlp",
                                       bufs=2)
                    nc.sync.dma_start(wch[:], d["w2"].ap()[gi])
                    for ml in range(2):
                        mt = gi * 2 + ml
                        ps = pool.tile([P, C], dt_f32, tag="mm")
                        for kt in range(FHT):
                            nc.tensor.matmul(
                                ps[:],
                                wch[:, (ml * FHT + kt) * P:
                                    (ml * FHT + kt + 1) * P],
                                h_sb[:, kt, :],
                                start=(kt == 0), stop=(kt == FHT - 1))
                        # drain on ACT (idle pre-basis window; DVE must
                        # stay clear for the post-AllReduce LN1 row math)
                        y = ystage.tile([P, C], dt_f32, tag="y")
                        nc.scalar.activation(
                            y[:], ps[:],
                            mybir.ActivationFunctionType.Identity,
                            bias=b2_sb[:, mt:mt + 1], scale=1.0)
                        nc.gpsimd.dma_start(
                            d["ym"].ap()[mt * P:(mt + 1) * P, :], y[:])

                for gi in range(4):
                    emit_mlp2_cols(gi, ps_m2)

            # ---- scope D: KAN L1, kt-outer, 8 resident PSUM banks; the
            # last chunk runs mt-inner so drains+output DMA overlap the
            # remaining matmuls instead of serializing at the end ----
            u1 = {}
            with tc.tile_pool(name="ps_l1", bufs=HT, space="PSUM") as ps_l1:
                psl = [ps_l1.tile([P, C], dt_f32, tag="l1", name=f"psl{mt}")
                       for mt in range(HT)]

                def basis1_for(kt):
                    ft = kt % F2HT
                    if ft not in u1:
                        u1[ft] = _emit_u_ft(nc, pools, z_sb, ft, br1,
                                            bm1, u_tag="u", bufs=F2HT,
                                            name=f"u1f{ft}")
                    return _emit_basis(nc, pools, u1[ft],
                                       cst_sb[:, SC1 + kt:SC1 + kt + 1],
                                       cst_sb[:, GB1 + kt:GB1 + kt + 1])

                def drain_yk(mt):
                    y = ystage.tile([P, C], dt_f32, tag="yfin", bufs=6)
                    if mt % 2 == 0:
                        nc.scalar.activation(
                            y[:], psl[mt][:],
                            mybir.ActivationFunctionType.Identity,
                            bias=bias1_sb[:, mt:mt + 1], scale=1.0)
                    else:
                        nc.vector.tensor_scalar_add(y[:], psl[mt][:],
                                                    bias1_sb[:, mt:mt + 1])
                    (nc.sync if mt % 2 == 0 else nc.gpsimd).dma_start(
                        d["yk"].ap()[mt * P:(mt + 1) * P, :], y[:])

                n_chunks = KT // 8
                for ci in range(n_chunks):
                    wch = wstream.tile([P, 8 * HT * P], dt_bf, tag="wkan",
                                       bufs=2)
                    nc.sync.dma_start(wch[:], d["w1k"].ap()[ci])
                    if ci < n_chunks - 1:
                        for ktl in range(8):
                            kt = ci * 8 + ktl
                            b = basis1_for(kt)
                            for mt in range(HT):
                                nc.tensor.matmul(
                                    psl[mt][:],
                                    wch[:, (ktl * HT + mt) * P:
                                        (ktl * HT + mt + 1) * P],
                                    b[:],
                                    start=(kt == 0), stop=False)
                    else:
                        bs = [basis1_for(ci * 8 + ktl) for ktl in range(8)]
                        for mt in range(HT):
                            for ktl in range(8):
                                nc.tensor.matmul(
                                    psl[mt][:],
                                    wch[:, (ktl * HT + mt) * P:
                                        (ktl * HT + mt + 1) * P],
                                    bs[ktl][:],
                                    start=False, stop=(ktl == 7))
                            drain_yk(mt)

    nc.compile()
    return nc


_program_cache = None


def _get_program():
    global _program_cache
    if _program_cache is None:
        _program_cache = _build_program()
    return _program_cache


# --------------------------------------------------------------------------
# host reference math for overflow tokens (capacity exceeded)
# --------------------------------------------------------------------------

def _host_expert(e, xs, ins):
    xs = xs.astype(np.float32)
    if e < E2:
        h = xs @ ins["mlp_W1"][e] + ins["mlp_b1"][e]
        import math
        erf = np.vectorize(math.erf)
        h = h * 0.5 * (1.0 + erf(h / np.sqrt(2.0)))
        return h @ ins["mlp_W2"][e] + ins["mlp_b2"][e]
    k = e - E2

    def ln(v, g, b):
        mu = v.mean(-1, keepdims=True)
        var = v.var(-1, keepdims=True)
        return (v - mu) / np.sqrt(var + LN_EPS) * g + b

    def rswaf(v):
        t = np.tanh((v[..., None] - GRID) * INV_DENOM)
        return (1.0 - t * t).reshape(v.shape[0], -1)

    h0 = ln(xs, ins["kan_ln_g0"][k], ins["kan_ln_b0"][k])
    z = rswaf(h0) @ ins["kan_sl_W0"][k] + ins["kan_sl_b0"][k]
    h1 = ln(z, ins["kan_ln_g1"][k], ins["kan_ln_b1"][k])
    return rswaf(h1) @ ins["kan_sl_W1"][k] + ins["kan_sl_b1"][k]


# --------------------------------------------------------------------------
# main entry
# --------------------------------------------------------------------------

def kernel(hidden_states, gate_W, mlp_W1, mlp_b1, mlp_W2, mlp_b2,
           kan_ln_g0, kan_ln_b0, kan_sl_W0, kan_sl_b0,
           kan_ln_g1, kan_ln_b1, kan_sl_W1, kan_sl_b1):
    ins = dict(mlp_W1=np.asarray(mlp_W1), mlp_b1=np.asarray(mlp_b1),
               mlp_W2=np.asarray(mlp_W2), mlp_b2=np.asarray(mlp_b2),
               kan_ln_g0=np.asarray(kan_ln_g0), kan_ln_b0=np.asarray(kan_ln_b0),
               kan_sl_W0=np.asarray(kan_sl_W0), kan_sl_b0=np.asarray(kan_sl_b0),
               kan_ln_g1=np.asarray(kan_ln_g1), kan_ln_b1=np.asarray(kan_ln_b1),
               kan_sl_W1=np.asarray(kan_sl_W1), kan_sl_b1=np.asarray(kan_sl_b1))
    hs = np.asarray(hidden_states)
    x = hs.reshape(T, H).astype(np.float32)

    _register_ntff_hook()

    # ---- route + shard (host side of the sharding strategy) ----
    sel, w_full = _route(x, np.asarray(gate_W))
    shards = []   # (expert, device_idx, overflow_idx) per expert
    for e in range(E):
        idx = np.nonzero(w_full[:, e] > 0)[0].astype(np.int64)
        shards.append((e, idx[:C], idx[C:]))

    def xpad_T(idx):
        # pad with a real token so every column has O(1) LN variance;
        # pre-tiled [P, HT, C] so the device DMA is fully contiguous
        fill = x[idx[0]] if len(idx) else x[0]
        out = np.broadcast_to(fill, (C, H)).copy()
        out[:len(idx)] = x[idx]
        xt = out.T.astype(BF16)
        return np.ascontiguousarray(xt.reshape(HT, P, C).transpose(1, 0, 2))

    xm_pair = [xpad_T(shards[e][1]) for e in range(E2)]
    xk_pair = [xpad_T(shards[E2 + k][1]) for k in range(E2)]

    mixed = [_mix_kan_expert(ins["kan_sl_W0"][p], ins["kan_sl_W1"][p])
             for p in range(E2)]
    in_maps = []
    for c in range(8):
        p, s = c // 2, c % 2
        mp = _prep_side_mlp(ins["mlp_W1"][p], ins["mlp_b1"][p],
                            ins["mlp_W2"][p], ins["mlp_b2"][p], s)
        kp = _prep_side_kan(ins["kan_ln_g0"][p], ins["kan_ln_b0"][p],
                            ins["kan_sl_b0"][p],
                            ins["kan_ln_g1"][p], ins["kan_ln_b1"][p],
                            ins["kan_sl_b1"][p], mixed[p], s)
        cst = np.concatenate([mp["b1"], mp["b2"], kp["bias0"], kp["bias1"],
                              kp["sc0"], kp["gb0"], kp["sc1"], kp["gb1"]],
                             axis=1)
        in_maps.append({
            "xm": xm_pair[p], "xk": xk_pair[p],
            "w1": mp["w1"], "w2": mp["w2"],
            "w0": kp["w0"], "w1k": kp["w1k"],
            "cst": np.ascontiguousarray(cst),
        })

    # ---- compile + run ----
    nc = _get_program()
    res = bass_utils.run_bass_kernel_spmd(nc, in_maps, core_ids=list(range(8)))
    last_run_info.clear()
    last_run_info.update(
        exec_time_ns=res.exec_time_ns,
        mean_exec_time_ns=res.mean_exec_time_ns,
        max_exec_time_core_id=res.max_exec_time_core_id,
        profile_json=res.profile_json,
        res=res,
    )

    # ---- host combine: out[t] = sum_e w[t,e] * (y_e_side0[t] + y_e_side1[t])
    out = np.zeros((T, H), np.float32)
    for p in range(E2):
        for (e, name) in ((p, "ym"), (E2 + p, "yk")):
            idx = shards[e][1]
            n = len(idx)
            if n:
                y = (res.results[2 * p][name].astype(np.float32)
                     + res.results[2 * p + 1][name].astype(np.float32))
                out[idx] += w_full[idx, e][:, None] * y[:, :n].T
    # overflow tokens (beyond capacity): exact host math
    for e, _idx, ovf in shards:
        if len(ovf):
            y = _host_expert(e, x[ovf], ins)
            out[ovf] += w_full[ovf, e][:, None] * y
    return out.reshape(hs.shape).astype(np.float32)



# revision 21
# speedup vs baseline: 1.0591x; 1.0591x over previous
"""MoE (4 MLP experts + 4 FasterKAN experts, top-2) Trainium2 kernel.

Sharding: expert-parallel, feature-split across core pairs. The router (tiny)
runs on the host as part of input sharding. Cores (2p, 2p+1) own MLP expert p
and KAN expert 4+p; each core processes ALL of its experts' routed tokens
(capacity 512; overflow handled exactly on host) but only HALF of each
expert's feature dimension:

  MLP:  core side s computes h = gelu(x @ W1[:, sF:sF+F/2]) and the partial
        y_s = h @ W2[sF:sF+F/2, :]; host sums y_0 + y_1. No cross-core traffic.
  KAN:  core side s computes z-half = basis0 @ W0[:, half] (layer-0 output
        features split) and layer-1 partial y_s from its own z-half's basis
        rows (K split); host sums partials. LayerNorm-1 needs mean/var over
        the FULL F2 features -> tiny [1,1024] fp32 AllReduce over the core
        pair (Σz | Σz² packed in one row), overlapped with the MLP phase.

Device numerics: all matmuls bf16 with fp32 PSUM accumulation, N=512 free
dim, kt-outer accumulation with ring-buffered RSWAF basis tiles (never fully
resident). LN column stats via PE ones-matmul; rstd via DVE reciprocal seed
+ Newton iterations. The G=8 RSWAF grid functions are compressed to NA=3
fitted sech^2 atoms + a constant (grid dim pre-mixed into the spline weights
on the host), shrinking the KAN K dim from G*H to NA*H; the constant term
and the "+1" of the -tanh^2 form fold into the output bias.
"""

import os

import numpy as np
import ml_dtypes

import concourse.bass as bass
import concourse.tile as tile
from concourse import bacc, mybir
from concourse import bass_utils

BF16 = ml_dtypes.bfloat16

# ---- problem constants (hardcoded per contract) ----
T, H, F, E = 2048, 1024, 4096, 8
F2 = F // 2
E2 = E // 2
G = 8
TOP_K = 2
INV_DENOM = 0.5
GRID = np.linspace(-1.2, 0.2, G).astype(np.float32)
LN_EPS = 1e-5
P = 128
C = 512            # capacity per expert (all tokens; overflow -> host)
HT = H // P        # 8 H-tiles
FH = F // 2        # 2048: MLP F half per core
FHT = FH // P      # 16
F2H = F2 // 2      # 1024: KAN z-feature half per core
F2HT = F2H // P    # 8

# ---- low-rank RSWAF basis compression ----
# The 8 grid basis functions sech^2((x-c_g)/2), c_g in linspace(-1.2,0.2,8),
# are approximated (N(0,1)-weighted LSQ, the post-LN input distribution) by
#   b_g(x) ~= COEF[0,g] + sum_r COEF[1+r,g] * sech^2(ATOM_A[r]*x + ATOM_B[r])
# with NA=3 fitted atoms (weighted RMS 1.2e-3; end-to-end rel err 1.6e-3).
# The grid dim is mixed into the spline weights on the host, so the device
# K dim shrinks from G*H to NA*H (2.67x less PE work for the KAN experts).
NA = 3
ATOM_A = np.array([0.5104, 0.4978, 0.4977], np.float64)
ATOM_B = np.array([0.2581, 0.5374, -0.0356], np.float64)
COEF = np.array([
    [-7.49781605e-03,  3.85725206e-03,  8.71658627e-03,  6.63497338e-03,
     -2.92025891e-04, -7.25583852e-03, -7.22697257e-03,  7.91052327e-03],
    [-3.26024659e-01,  2.25285923e-01,  6.30653143e-01,  8.36926569e-01,
      8.18689331e-01,  5.85328647e-01,  1.81687157e-01, -3.18349195e-01],
    [ 1.25153196e+00,  8.26446110e-01,  4.54749619e-01,  1.71840623e-01,
     -1.45976379e-03, -6.35530915e-02, -3.30161903e-02,  5.46003779e-02],
    [ 7.34034472e-02, -5.47935817e-02, -8.74939226e-02, -4.93450897e-03,
      1.95608070e-01,  4.96872627e-01,  8.62470677e-01,  1.24173739e+00],
], np.float64)

KT = NA * 8        # 24 K-tiles for both KAN layers per core (3*1024 / 128)

last_run_info = {}


def _register_ntff_hook():
    """Best-effort NTFF profiling hook registration (used when BASS_TRACE=1)."""
    try:
        import sys
        import types
        try:
            from antenv import axon_hooks  # noqa: F401
        except ImportError:
            # the image's antenv lacks axon_hooks; install a functional shim
            # so bass_utils' `from antenv.axon_hooks import ...` resolves
            import antenv
            mod = types.ModuleType("antenv.axon_hooks")
            mod._hook = None
            mod.set_axon_ntff_profile_hook = \
                lambda h, _m=mod: setattr(_m, "_hook", h)
            mod.get_axon_ntff_profile_hook = lambda _m=mod: _m._hook
            antenv.axon_hooks = mod
            sys.modules["antenv.axon_hooks"] = mod
        from antenv.axon_hooks import set_axon_ntff_profile_hook, \
            get_axon_ntff_profile_hook
        if get_axon_ntff_profile_hook() is not None:
            return
        from trn_agent_boot.trn_boot import _ntff_profile_via_ctypes
        so = "/opt/axon/libaxon_pjrt.so"
        if os.path.exists(so):
            set_axon_ntff_profile_hook(_ntff_profile_via_ctypes(so))
            # artifact upload needs a cloud bucket; keep artifacts local
            bass_utils.upload_artifacts = lambda tmpdir: tmpdir
    except Exception:
        pass


# --------------------------------------------------------------------------
# host-side routing (the dispatch half of the sharding strategy)
# --------------------------------------------------------------------------

def _route(x, gate_w):
    """Replicates the reference router in fp32. Returns (sel, w_full)."""
    logits = x.astype(np.float32) @ gate_w.astype(np.float32)        # [T, E]
    m = logits.max(axis=-1, keepdims=True)
    p = np.exp(logits - m, dtype=np.float32)
    probs = p / p.sum(axis=-1, keepdims=True, dtype=np.float32)
    # jax.lax.top_k semantics: descending, ties -> lower index first
    sel = np.argsort(-probs, axis=-1, kind="stable")[:, :TOP_K]      # [T, K]
    rw = np.take_along_axis(probs, sel, axis=-1)
    rw = rw / rw.sum(axis=-1, keepdims=True)
    w_full = np.zeros((T, E), np.float32)
    np.put_along_axis(w_full, sel, rw.astype(np.float32), axis=-1)
    return sel, w_full


# --------------------------------------------------------------------------
# host-side weight pre-tiling
# --------------------------------------------------------------------------

def _pretile_grouped(w, n_kt, n_mt, group):
    """[K, M] fp32 -> [n_mt/group, P, group*n_kt*P] bf16:
    out[gi, kp, ml*n_kt*P + kt*P + m] = w[kt*P+kp, (gi*group+ml)*P+m]."""
    a = w.reshape(n_kt, P, n_mt, P).transpose(2, 1, 0, 3)    # [mt, kp, kt, m]
    a = a.reshape(n_mt // group, group, P, n_kt, P).transpose(0, 2, 1, 3, 4)
    return np.ascontiguousarray(
        a.reshape(n_mt // group, P, group * n_kt * P).astype(BF16))


def _pretile_ktmajor(w, n_kt, n_mt, group):
    """kt-major: out[ci, kp, ktl*n_mt*P + mt*P + m] = w[(ci*group+ktl)*P+kp,
    mt*P+m] — one chunk holds `group` consecutive K-tiles across all mt."""
    a = w.reshape(n_kt // group, group, P, n_mt * P)         # [ci, ktl, kp, M]
    a = a.transpose(0, 2, 1, 3)
    return np.ascontiguousarray(
        a.reshape(n_kt // group, P, group * n_mt * P).astype(BF16))


def _pack_pp(v):
    """[n*P] fp32 per-feature vector -> [P, n] (partition-major) fp32."""
    n = v.shape[0] // P
    return np.ascontiguousarray(v.reshape(n, P).T.astype(np.float32))


def _prep_side_mlp(w1, b1, w2, b2, s):
    """Feature-half s of one MLP expert."""
    lo, hi = s * FH, (s + 1) * FH
    w1h = w1[:, lo:hi]                               # [H, FH]
    w2h = w2[lo:hi, :]                               # [FH, H]
    b2e = b2 if s == 0 else np.zeros_like(b2)
    return {
        "w1": _pretile_grouped(w1h, HT, FHT, 4),     # [4, 128, 4096]
        "w2": _pretile_grouped(w2h, FHT, HT, 2),     # [4, 128, 4096]
        "b1": _pack_pp(b1[lo:hi]),                   # [128, 16]
        "b2": _pack_pp(b2e),                         # [128, 8]
    }


def _mix_kan_expert(w0, w1):
    """Mix the grid dim of one KAN expert's spline weights with the fitted
    atom coefficients (once per expert; both sides slice the result).

    Returns (w0r [NA*H, F2], c0 [F2], w1g [F2, G, H], w1r [NA*F2, H]) where
    w0r/w1r rows are atom-major (atom r block, then feature) and c0 is the
    layer-0 constant-term fold COEF[0] applied to the grid dim."""
    cf = COEF.astype(np.float32)
    w0g = w0.reshape(H, G, F2)                       # rows (h, g)
    w0r = np.einsum('rg,hgf->rhf', cf[1:], w0g).reshape(NA * H, F2)
    c0 = COEF[0] @ w0g.sum(0, dtype=np.float64)      # [F2]
    w1g = w1.reshape(F2, G, H)
    w1r = np.einsum('rg,fgh->rfh', cf[1:], w1g).reshape(NA * F2, H)
    return w0r, c0, w1g, w1r


def _prep_side_kan(g0, b0, sb0, g1, b1, sb1, mixed, s):
    """Feature-half s of one KAN expert (z features / layer-1 K rows)."""
    w0r, c0, w1g, w1r = mixed
    lo, hi = s * F2H, (s + 1) * F2H
    w0h = w0r[:, lo:hi]                              # [3H, F2H]
    # layer 1: atom-major rows for OWN z-half features
    w1h = w1r.reshape(NA, F2, H)[:, lo:hi].reshape(NA * F2H, H)
    # bias fold: spline bias + const-term fold + "+1" of the -tanh^2 form
    bias0 = (sb0[lo:hi].astype(np.float64) + c0[lo:hi]
             + w0h.astype(np.float64).sum(0)).astype(np.float32)
    sb1e = sb1 if s == 0 else np.zeros_like(sb1)
    bias1 = (sb1e.astype(np.float64)
             + COEF[0] @ w1g[lo:hi].sum(0, dtype=np.float64)
             + w1h.astype(np.float64).sum(0)).astype(np.float32)
    # tanh scale/bias tables, col kt = r*8 + ft:
    #   tanh(a_r*(gamma*u + beta) + b_r) -> scale = a_r*gamma, bias = a_r*beta + b_r
    g0p, b0p = _pack_pp(g0), _pack_pp(b0)            # [128, 8]
    g1p, b1p = _pack_pp(g1[lo:hi]), _pack_pp(b1[lo:hi])
    sc0 = np.concatenate([ATOM_A[r] * g0p for r in range(NA)], 1)
    gb0 = np.concatenate([ATOM_A[r] * b0p + ATOM_B[r] for r in range(NA)], 1)
    sc1 = np.concatenate([ATOM_A[r] * g1p for r in range(NA)], 1)
    gb1 = np.concatenate([ATOM_A[r] * b1p + ATOM_B[r] for r in range(NA)], 1)
    return {
        # negated: the device accumulates +tanh^2 tiles (see _emit_basis)
        "w0": _pretile_ktmajor(-w0h, KT, F2HT, 8),   # [3, 128, 8192]
        "w1k": _pretile_ktmajor(-w1h, KT, HT, 8),    # [3, 128, 8192]
        "bias0": _pack_pp(bias0),                    # [128, 8]
        "bias1": _pack_pp(bias1),                    # [128, 8]
        "sc0": np.ascontiguousarray(sc0.astype(np.float32)),   # [128, 24]
        "gb0": np.ascontiguousarray(gb0.astype(np.float32)),
        "sc1": np.ascontiguousarray(sc1.astype(np.float32)),
        "gb1": np.ascontiguousarray(gb1.astype(np.float32)),
    }


# --------------------------------------------------------------------------
# device program
# --------------------------------------------------------------------------

def _emit_stat_ft(nc, pools, x_sb, ft, n_ft, psx, psx2, ones_sb,
                  square_on_act=False):
    """One feature tile's contribution to column mean / mean-square.

    ones_sb carries 1/D so PSUM accumulates E[x] and E[x^2] directly.
    square_on_act routes the elementwise square to the scalar engine
    (Square shares every ACT table) when the DVE is the busier engine."""
    sbuf = pools["sbuf"]
    x2 = sbuf.tile([P, C], mybir.dt.bfloat16, tag="x2")
    if square_on_act:
        nc.scalar.activation(x2[:], x_sb[:, ft, :],
                             mybir.ActivationFunctionType.Square)
    else:
        nc.vector.tensor_tensor(x2[:], x_sb[:, ft, :], x_sb[:, ft, :],
                                op=mybir.AluOpType.mult)
    nc.tensor.matmul(psx[:], ones_sb[:], x_sb[:, ft, :],
                     start=(ft == 0), stop=(ft == n_ft - 1))
    nc.tensor.matmul(psx2[:], ones_sb[:], x2[:],
                     start=(ft == 0), stop=(ft == n_ft - 1))


def _emit_ln_rows(nc, pools, mean_ap, ex2_ap):
    """Row math: (E[x], E[x^2]) -> (rstd, -mu*rstd) as bf16 [1, C] rows.

    rstd = rsqrt(var + eps) entirely on DVE: quake-style magic seed
    (0x5f3759df) + one Newton iteration (rel err ~2e-3, far inside the
    bf16 downstream precision). Avoids ACT Sqrt so the scalar engine
    never swaps activation tables mid-kernel.
    """
    rows = pools["rows"]
    f32, u32 = mybir.dt.float32, mybir.dt.uint32
    var = rows.tile([1, C], f32, tag="row")
    t = rows.tile([1, C], f32, tag="row")
    r0 = rows.tile([1, C], u32, tag="row")
    rstd = rows.tile([1, C], mybir.dt.bfloat16, tag="rowb")
    negmr = rows.tile([1, C], mybir.dt.bfloat16, tag="rowb")
    if mean_ap.space == bass.MemorySpace.PSUM:
        # ops may read at most one non-scalar PSUM input
        mcopy = rows.tile([1, C], f32, tag="row")
        nc.vector.tensor_scalar_mul(mcopy[:], mean_ap, 1.0)
        mean_ap = mcopy[:]
    nc.vector.scalar_tensor_tensor(t[:], mean_ap, -1.0, mean_ap,
                                   op0=mybir.AluOpType.mult,
                                   op1=mybir.AluOpType.mult)     # -mean^2
    nc.vector.scalar_tensor_tensor(var[:], t[:], LN_EPS, ex2_ap,
                                   op0=mybir.AluOpType.add,
                                   op1=mybir.AluOpType.add)      # var + eps
    nc.vector.tensor_scalar(r0[:], var[:].bitcast(u32), 1, None,
                            op0=mybir.AluOpType.logical_shift_right)
    # magic - s without u32 wraparound (DVE arith may run via fp32)
    nc.vector.scalar_tensor_tensor(r0[:], pools["magic"][:].bitcast(u32),
                                   1.0, r0[:],
                                   op0=mybir.AluOpType.mult,
                                   op1=mybir.AluOpType.subtract)
    rf = r0[:].bitcast(f32)
    nc.vector.tensor_tensor(t[:], rf, rf, op=mybir.AluOpType.mult)
    nc.vector.tensor_tensor(t[:], t[:], var[:], op=mybir.AluOpType.mult)
    nc.vector.tensor_scalar(t[:], t[:], -0.5, 1.5,
                            op0=mybir.AluOpType.mult,
                            op1=mybir.AluOpType.add)             # 1.5-.5vr^2
    nc.vector.tensor_tensor(rstd[:], rf, t[:], op=mybir.AluOpType.mult)
    nc.vector.scalar_tensor_tensor(negmr[:], mean_ap, -1.0, rstd[:],
                                   op0=mybir.AluOpType.mult,
                                   op1=mybir.AluOpType.mult)     # -mu*rstd
    return rstd, negmr


def _emit_ln_bcast(nc, pools, psum_bc, rstd, negmr):
    """Per-layer [P, C] broadcasts of rstd and -mu*rstd (PE rank-1 outer with
    a bf16 ones row, drained to bf16 SBUF)."""
    bvec = pools["bvec"]
    onesf = pools["onesf"]
    br_ps = psum_bc.tile([P, C], mybir.dt.float32, tag="bcast")
    bm_ps = psum_bc.tile([P, C], mybir.dt.float32, tag="bcast")
    nc.tensor.matmul(br_ps[:], onesf[:], rstd[:], start=True, stop=True)
    nc.tensor.matmul(bm_ps[:], onesf[:], negmr[:], start=True, stop=True)
    br = bvec.tile([P, C], mybir.dt.bfloat16, tag="bvec")
    bm = bvec.tile([P, C], mybir.dt.bfloat16, tag="bvec")
    nc.scalar.activation(br[:], br_ps[:], mybir.ActivationFunctionType.Identity)
    nc.scalar.activation(bm[:], bm_ps[:], mybir.ActivationFunctionType.Identity)
    return br, bm


def _emit_u_ft(nc, pools, x_sb, ft, br, bm, u_tag, bufs, name=None):
    """u = x * br + bm (the LN affine transform is folded into the tanh)."""
    sbuf = pools["sbuf"]
    u = sbuf.tile([P, C], mybir.dt.bfloat16, tag=u_tag, bufs=bufs,
                  name=name or "u")
    nc.vector.tensor_tensor(u[:], x_sb[:, ft, :], br[:],
                            op=mybir.AluOpType.mult)
    nc.vector.tensor_tensor(u[:], u[:], bm[:], op=mybir.AluOpType.add)
    return u


def _emit_basis(nc, pools, u, scale_ap, bias_ap, tag="bas"):
    """ring tile = +tanh^2(a_r*(gamma*u + beta) + b_r).

    The atom weight blocks are negated on the host so the accumulated
    sign comes out right; the "+1" of (1 - tanh^2) lives in the bias."""
    sbuf = pools["sbuf"]
    th = sbuf.tile([P, C], mybir.dt.bfloat16, tag="th", bufs=3)
    nc.scalar.activation(th[:], u[:],
                         mybir.ActivationFunctionType.Tanh,
                         bias=bias_ap, scale=scale_ap)
    b = sbuf.tile([P, C], mybir.dt.bfloat16, tag=tag, bufs=12)
    nc.vector.tensor_tensor(b[:], th[:], th[:], op=mybir.AluOpType.mult)
    return b


def _build_program():
    nc = bacc.Bacc("TRN2", target_bir_lowering=False, debug=False,
                   num_devices=8)
    dt_bf = mybir.dt.bfloat16
    dt_f32 = mybir.dt.float32

    d = {}
    d["xm"] = nc.dram_tensor("xm", [P, HT, C], dt_bf, kind="ExternalInput")
    d["xk"] = nc.dram_tensor("xk", [P, HT, C], dt_bf, kind="ExternalInput")
    d["w1"] = nc.dram_tensor("w1", [FHT // 4, P, 4 * HT * P], dt_bf,
                             kind="ExternalInput")
    d["w2"] = nc.dram_tensor("w2", [HT // 2, P, 2 * FHT * P], dt_bf,
                             kind="ExternalInput")
    d["w0"] = nc.dram_tensor("w0", [KT // 8, P, 8 * F2HT * P], dt_bf,
                             kind="ExternalInput")
    d["w1k"] = nc.dram_tensor("w1k", [KT // 8, P, 8 * HT * P], dt_bf,
                              kind="ExternalInput")
    # packed consts [P, 136]: b1(16) b2(8) bias0(8) bias1(8)
    #   sc0(24) gb0(24) sc1(24) gb1(24)  (tanh scale/bias, col = r*8+ft)
    d["cst"] = nc.dram_tensor("cst", [P, 136], dt_f32, kind="ExternalInput")
    d["ym"] = nc.dram_tensor("ym", [H, C], dt_f32, kind="ExternalOutput")
    d["yk"] = nc.dram_tensor("yk", [H, C], dt_f32, kind="ExternalOutput")

    with tile.TileContext(nc) as tc:
        with (
            tc.tile_pool(name="const", bufs=1) as const,
            tc.tile_pool(name="acts", bufs=1) as acts,
            tc.tile_pool(name="work", bufs=3) as work,
            tc.tile_pool(name="bvecp", bufs=4) as bvecp,
            tc.tile_pool(name="wstream", bufs=4) as wstream,
            tc.tile_pool(name="rows", bufs=5) as rows,
            tc.tile_pool(name="ystage", bufs=3) as ystage,
            tc.tile_pool(name="dram", bufs=1, space="DRAM") as dram,
        ):
            # ---- input/const DMAs (xk first: LN0 gates the KAN pipeline) ----
            xk_sb = acts.tile([P, HT, C], dt_bf)
            nc.sync.dma_start(xk_sb[:], d["xk"].ap())
            xm_sb = acts.tile([P, HT, C], dt_bf)
            nc.sync.dma_start(xm_sb[:], d["xm"].ap())
            cst_sb = const.tile([P, 136], dt_f32)
            nc.gpsimd.dma_start(cst_sb[:], d["cst"].ap())
            b1_sb = cst_sb[:, 0:16]
            b2_sb = cst_sb[:, 16:24]
            bias0_sb = cst_sb[:, 24:32]
            bias1_sb = cst_sb[:, 32:40]
            SC0, GB0, SC1, GB1 = 40, 64, 88, 112

            ones0_sb = const.tile([P, 1], dt_bf)     # 1/H for LN0 stats
            nc.vector.memset(ones0_sb[:], 1.0 / H)
            ones1_sb = const.tile([P, 1], dt_bf)     # 1/F2 for LN1 stats
            nc.vector.memset(ones1_sb[:], 1.0 / F2)
            onesf_sb = const.tile([1, P], dt_bf)
            nc.vector.memset(onesf_sb[:], 1.0)
            # fp32 value whose bit pattern is the rsqrt magic 0x5F3759DF
            magic_sb = const.tile([1, C], dt_f32)
            nc.vector.memset(magic_sb[:], 1.3211836172961055e+19)

            pools = {"sbuf": work, "rows": rows,
                     "onesf": onesf_sb, "bvec": bvecp, "magic": magic_sb}

            # table preloads: dummy Gelu+Tanh force the ACT table load(s)
            # during the input-DMA dead time instead of mid-pipeline
            tw_sb = const.tile([1, 2], dt_bf)
            nc.scalar.activation(tw_sb[:, 0:1], magic_sb[0:1, 0:1],
                                 mybir.ActivationFunctionType.Gelu)
            nc.scalar.activation(tw_sb[:, 1:2], magic_sb[0:1, 0:1],
                                 mybir.ActivationFunctionType.Tanh)
            # PE warm burst: ~4us of tiny matmuls un-throttle the HAM clock
            # gate before the first real matmuls arrive
            wz_sb = const.tile([P, 64], dt_bf)
            nc.vector.memset(wz_sb[:], 0.0)

            h_sb = acts.tile([P, FHT, C], dt_bf)     # MLP hidden (GELU'd)
            z_sb = acts.tile([P, F2HT, C], dt_bf)    # KAN z half

            # stats packed row for the pair AllReduce: [Σz | Σz²]
            sums_sb = rows.tile([1, 2 * C], dt_f32, name="sums")
            sums2_sb = rows.tile([1, 2 * C], dt_f32, name="sums2")
            cc_in = dram.tile([1, 2 * C], dt_f32)
            cc_out = dram.tile([1, 2 * C], dt_f32)

            # warmup collective: absorbs the ncfw control-plane startup cost
            # (~40-75us) so the real stats AllReduce later completes fast.
            # Input is an uninitialized internal scratch tile (summed garbage
            # is never read) so the doorbell carries no DMA dependency and
            # never blocks the gpsimd queue.
            ccw_in = dram.tile([1, P], dt_f32)
            ccw_out = dram.tile([1, P], dt_f32)
            nc.gpsimd.collective_compute(
                "AllReduce", mybir.AluOpType.add,
                replica_groups=[[0, 1], [2, 3], [4, 5], [6, 7]],
                ins=[ccw_in.opt()], outs=[ccw_out.opt()])

            # ---- scope A: LN0 stats + bcast; MLP L1 first group ----
            with (
                tc.tile_pool(name="ps_stat0", bufs=2, space="PSUM") as ps_s0,
                tc.tile_pool(name="ps_bc0", bufs=2, space="PSUM") as ps_b0,
                tc.tile_pool(name="ps_mlp1", bufs=3, space="PSUM") as ps_m1,
                tc.tile_pool(name="ps_warm", bufs=1, space="PSUM") as ps_w,
            ):
                pswm = ps_w.tile([64, 64], dt_f32, tag="warm")
                for _ in range(64):
                    nc.tensor.matmul(pswm[:], wz_sb[:], wz_sb[:],
                                     start=True, stop=True)
                psx = ps_s0.tile([1, C], dt_f32, tag="stat")
                psx2 = ps_s0.tile([1, C], dt_f32, tag="stat")
                for ft in range(HT):
                    _emit_stat_ft(nc, pools, xk_sb, ft, HT, psx, psx2,
                                  ones0_sb)
                rstd0, negmr0 = _emit_ln_rows(nc, pools, psx[0:1, :],
                                              psx2[0:1, :])
                # bcast + the whole chunk-0 basis prep emitted BEFORE the
                # MLP groups: the 8 tanh ops land ahead of the GELU drains
                # in the ACT FIFO and complete behind the MLP matmuls, so
                # KAN L0 starts with all of chunk 0 ready
                br0, bm0 = _emit_ln_bcast(nc, pools, ps_b0, rstd0, negmr0)
                u0 = {}

                def basis0_for(kt):
                    ft = kt % HT
                    if ft not in u0:
                        u0[ft] = _emit_u_ft(nc, pools, xk_sb, ft, br0,
                                            bm0, u_tag="u", bufs=HT,
                                            name=f"u0f{ft}")
                    return _emit_basis(nc, pools, u0[ft],
                                       cst_sb[:, SC0 + kt:SC0 + kt + 1],
                                       cst_sb[:, GB0 + kt:GB0 + kt + 1])

                bs0 = [basis0_for(ktl) for ktl in range(8)]

                # ---- MLP L1 groups 0-1 (fill the PE while LN0 ramps) ----
                def drain_h(mt, ps):
                    nc.scalar.activation(h_sb[:, mt, :], ps[:],
                                         mybir.ActivationFunctionType.Gelu,
                                         bias=b1_sb[:, mt:mt + 1], scale=1.0)

                def emit_mlp1_group(gi, pool, q):
                    wch = wstream.tile([P, 4 * HT * P], dt_bf, tag="wmlp",
                                       bufs=2)
                    q.dma_start(wch[:], d["w1"].ap()[gi])
                    for ml in range(4):
                        mt = gi * 4 + ml
                        ps = pool.tile([P, C], dt_f32, tag="mm")
                        for kt in range(HT):
                            nc.tensor.matmul(
                                ps[:],
                                wch[:, (ml * HT + kt) * P:
                                    (ml * HT + kt + 1) * P],
                                xm_sb[:, kt, :],
                                start=(kt == 0), stop=(kt == HT - 1))
                        drain_h(mt, ps)

                # sync queue: the gpsimd queue is blocked until ~20us by
                # the warmup-collective trigger (ncfw boot), which would
                # stall these weight DMAs and with them the whole phase
                emit_mlp1_group(0, ps_m1, nc.sync)
                emit_mlp1_group(1, ps_m1, nc.sync)

            # ---- scope B: KAN L0, kt-outer, 8 resident PSUM banks.  The
            # last weight chunk runs mt-inner so the banks complete (and
            # drain) staggered instead of all at once at kt==KT-1. ----
            with tc.tile_pool(name="ps_l0", bufs=F2HT, space="PSUM") as ps_l0:
                psz = [ps_l0.tile([P, C], dt_f32, tag="l0", name=f"psz{mt}")
                       for mt in range(F2HT)]

                def drain_z(mt):
                    if mt % 2 == 0:
                        nc.scalar.activation(
                            z_sb[:, mt, :], psz[mt][:],
                            mybir.ActivationFunctionType.Identity,
                            bias=bias0_sb[:, mt:mt + 1], scale=1.0)
                    else:
                        nc.vector.tensor_scalar_add(
                            z_sb[:, mt, :], psz[mt][:],
                            bias0_sb[:, mt:mt + 1])

                n_chunks = KT // 8
                for ci in range(n_chunks):
                    wch = wstream.tile([P, 8 * F2HT * P], dt_bf, tag="wkan",
                                       bufs=2)
                    nc.sync.dma_start(wch[:], d["w0"].ap()[ci])
                    if ci < n_chunks - 1:
                        for ktl in range(8):
                            kt = ci * 8 + ktl
                            b = bs0[ktl] if ci == 0 else basis0_for(kt)
                            for mt in range(F2HT):
                                nc.tensor.matmul(
                                    psz[mt][:],
                                    wch[:, (ktl * F2HT + mt) * P:
                                        (ktl * F2HT + mt + 1) * P],
                                    b[:],
                                    start=(kt == 0), stop=False)
                    else:
                        bs = [basis0_for(ci * 8 + ktl) for ktl in range(8)]
                        for mt in range(F2HT):
                            for ktl in range(8):
                                nc.tensor.matmul(
                                    psz[mt][:],
                                    wch[:, (ktl * F2HT + mt) * P:
                                        (ktl * F2HT + mt + 1) * P],
                                    bs[ktl][:],
                                    start=False, stop=(ktl == 7))
                            drain_z(mt)

            # ---- LN1 partial stats -> pack -> AllReduce over the pair ----
            with tc.tile_pool(name="ps_stat1", bufs=2, space="PSUM") as ps_s1:
                psx1 = ps_s1.tile([1, C], dt_f32, tag="stat")
                psx21 = ps_s1.tile([1, C], dt_f32, tag="stat")
                for mt in range(F2HT):
                    _emit_stat_ft(nc, pools, z_sb, mt, F2HT, psx1, psx21,
                                  ones1_sb)
                nc.vector.tensor_scalar_mul(sums_sb[:, 0:C], psx1[:], 1.0)
                nc.vector.tensor_scalar_mul(sums_sb[:, C:2 * C], psx21[:], 1.0)
            nc.gpsimd.dma_start(cc_in[:], sums_sb[:])
            nc.gpsimd.collective_compute(
                "AllReduce", mybir.AluOpType.add,
                replica_groups=[[0, 1], [2, 3], [4, 5], [6, 7]],
                ins=[cc_in.opt()], outs=[cc_out.opt()])

            # ---- scope C: MLP L1 g2-3, LN1 chain, MLP L2 (all 8 cols) ----
            with (
                tc.tile_pool(name="ps_mlp1b", bufs=2, space="PSUM") as ps_m1b,
                tc.tile_pool(name="ps_mlp2", bufs=2, space="PSUM") as ps_m2,
                tc.tile_pool(name="ps_bc1", bufs=2, space="PSUM") as ps_b1,
            ):
                emit_mlp1_group(2, ps_m1b, nc.gpsimd)
                emit_mlp1_group(3, ps_m1b, nc.gpsimd)

                # LN1 chain emitted BEFORE MLP L2 so its ACT/DVE ops are
                # prioritized ahead of the ym drains in the strict FIFOs
                nc.gpsimd.dma_start(sums2_sb[:], cc_out[:])
                rstd1, negmr1 = _emit_ln_rows(nc, pools,
                                              sums2_sb[0:1, 0:C],
                                              sums2_sb[0:1, C:2 * C])
                br1, bm1 = _emit_ln_bcast(nc, pools, ps_b1, rstd1, negmr1)

                def emit_mlp2_cols(gi, pool):
                    wch = wstream.tile([P, 2 * FHT * P], dt_bf, tag="wmlp",
                                       bufs=2)
                    nc.sync.dma_start(wch[:], d["w2"].ap()[gi])
                    for ml in range(2):
                        mt = gi * 2 + ml
                        ps = pool.tile([P, C], dt_f32, tag="mm")
                        for kt in range(FHT):
                            nc.tensor.matmul(
                                ps[:],
                                wch[:, (ml * FHT + kt) * P:
                                    (ml * FHT + kt + 1) * P],
                                h_sb[:, kt, :],
                                start=(kt == 0), stop=(kt == FHT - 1))
                        # drain on ACT (idle pre-basis window; DVE must
                        # stay clear for the post-AllReduce LN1 row math)
                        y = ystage.tile([P, C], dt_f32, tag="y")
                        nc.scalar.activation(
                            y[:], ps[:],
                            mybir.ActivationFunctionType.Identity,
                            bias=b2_sb[:, mt:mt + 1], scale=1.0)
                        nc.gpsimd.dma_start(
                            d["ym"].ap()[mt * P:(mt + 1) * P, :], y[:])

                for gi in range(4):
                    emit_mlp2_cols(gi, ps_m2)

            # ---- scope D: KAN L1, kt-outer, 8 resident PSUM banks; the
            # last chunk runs mt-inner so drains+output DMA overlap the
            # remaining matmuls instead of serializing at the end ----
            u1 = {}
            with tc.tile_pool(name="ps_l1", bufs=HT, space="PSUM") as ps_l1:
                psl = [ps_l1.tile([P, C], dt_f32, tag="l1", name=f"psl{mt}")
                       for mt in range(HT)]

                def basis1_for(kt):
                    ft = kt % F2HT
                    if ft not in u1:
                        u1[ft] = _emit_u_ft(nc, pools, z_sb, ft, br1,
                                            bm1, u_tag="u", bufs=F2HT,
                                            name=f"u1f{ft}")
                    return _emit_basis(nc, pools, u1[ft],
                                       cst_sb[:, SC1 + kt:SC1 + kt + 1],
                                       cst_sb[:, GB1 + kt:GB1 + kt + 1])

                def drain_yk(mt):
                    y = ystage.tile([P, C], dt_f32, tag="yfin", bufs=6)
                    if mt % 2 == 0:
                        nc.scalar.activation(
                            y[:], psl[mt][:],
                            mybir.ActivationFunctionType.Identity,
                            bias=bias1_sb[:, mt:mt + 1], scale=1.0)
                    else:
                        nc.vector.tensor_scalar_add(y[:], psl[mt][:],
                                                    bias1_sb[:, mt:mt + 1])
                    (nc.sync if mt % 2 == 0 else nc.gpsimd).dma_start(
                        d["yk"].ap()[mt * P:(mt + 1) * P, :], y[:])

                n_chunks = KT // 8
                for ci in range(n_chunks):
                    wch = wstream.tile([P, 8 * HT * P], dt_bf, tag="wkan",
                                       bufs=2)
                    nc.sync.dma_start(wch[:], d["w1k"].ap()[ci])
                    if ci < n_chunks - 1:
                        for ktl in range(8):
                            kt = ci * 8 + ktl
                            b = basis1_for(kt)
                            for mt in range(HT):
                                nc.tensor.matmul(
                                    psl[mt][:],
                                    wch[:, (ktl * HT + mt) * P:
                                        (ktl * HT + mt + 1) * P],
                                    b[:],
                                    start=(kt == 0), stop=False)
                    else:
                        bs = [basis1_for(ci * 8 + ktl) for ktl in range(8)]
                        for mt in range(HT):
                            for ktl in range(8):
                                nc.tensor.matmul(
                                    psl[mt][:],
                                    wch[:, (ktl * HT + mt) * P:
                                        (ktl * HT + mt + 1) * P],
                                    bs[ktl][:],
                                    start=False, stop=(ktl == 7))
                            drain_yk(mt)

    nc.compile()
    return nc


_program_cache = None


def _get_program():
    global _program_cache
    if _program_cache is None:
        _program_cache = _build_program()
    return _program_cache


# --------------------------------------------------------------------------
# host reference math for overflow tokens (capacity exceeded)
# --------------------------------------------------------------------------

def _host_expert(e, xs, ins):
    xs = xs.astype(np.float32)
    if e < E2:
        h = xs @ ins["mlp_W1"][e] + ins["mlp_b1"][e]
        import math
        erf = np.vectorize(math.erf)
        h = h * 0.5 * (1.0 + erf(h / np.sqrt(2.0)))
        return h @ ins["mlp_W2"][e] + ins["mlp_b2"][e]
    k = e - E2

    def ln(v, g, b):
        mu = v.mean(-1, keepdims=True)
        var = v.var(-1, keepdims=True)
        return (v - mu) / np.sqrt(var + LN_EPS) * g + b

    def rswaf(v):
        t = np.tanh((v[..., None] - GRID) * INV_DENOM)
        return (1.0 - t * t).reshape(v.shape[0], -1)

    h0 = ln(xs, ins["kan_ln_g0"][k], ins["kan_ln_b0"][k])
    z = rswaf(h0) @ ins["kan_sl_W0"][k] + ins["kan_sl_b0"][k]
    h1 = ln(z, ins["kan_ln_g1"][k], ins["kan_ln_b1"][k])
    return rswaf(h1) @ ins["kan_sl_W1"][k] + ins["kan_sl_b1"][k]


# --------------------------------------------------------------------------
# main entry
# --------------------------------------------------------------------------

def kernel(hidden_states, gate_W, mlp_W1, mlp_b1, mlp_W2, mlp_b2,
           kan_ln_g0, kan_ln_b0, kan_sl_W0, kan_sl_b0,
           kan_ln_g1, kan_ln_b1, kan_sl_W1, kan_sl_b1):
    ins = dict(mlp_W1=np.asarray(mlp_W1), mlp_b1=np.asarray(mlp_b1),
               mlp_W2=np.asarray(mlp_W2), mlp_b2=np.asarray(mlp_b2),
               kan_ln_g0=np.asarray(kan_ln_g0), kan_ln_b0=np.asarray(kan_ln_b0),
               kan_sl_W0=np.asarray(kan_sl_W0), kan_sl_b0=np.asarray(kan_sl_b0),
               kan_ln_g1=np.asarray(kan_ln_g1), kan_ln_b1=np.asarray(kan_ln_b1),
               kan_sl_W1=np.asarray(kan_sl_W1), kan_sl_b1=np.asarray(kan_sl_b1))
    hs = np.asarray(hidden_states)
    x = hs.reshape(T, H).astype(np.float32)

    _register_ntff_hook()

    # ---- route + shard (host side of the sharding strategy) ----
    sel, w_full = _route(x, np.asarray(gate_W))
    shards = []   # (expert, device_idx, overflow_idx) per expert
    for e in range(E):
        idx = np.nonzero(w_full[:, e] > 0)[0].astype(np.int64)
        shards.append((e, idx[:C], idx[C:]))

    def xpad_T(idx):
        # pad with a real token so every column has O(1) LN variance;
        # pre-tiled [P, HT, C] so the device DMA is fully contiguous
        fill = x[idx[0]] if len(idx) else x[0]
        out = np.broadcast_to(fill, (C, H)).copy()
        out[:len(idx)] = x[idx]
        xt = out.T.astype(BF16)
        return np.ascontiguousarray(xt.reshape(HT, P, C).transpose(1, 0, 2))

    xm_pair = [xpad_T(shards[e][1]) for e in range(E2)]
    xk_pair = [xpad_T(shards[E2 + k][1]) for k in range(E2)]

    mixed = [_mix_kan_expert(ins["kan_sl_W0"][p], ins["kan_sl_W1"][p])
             for p in range(E2)]
    in_maps = []
    for c in range(8):
        p, s = c // 2, c % 2
        mp = _prep_side_mlp(ins["mlp_W1"][p], ins["mlp_b1"][p],
                            ins["mlp_W2"][p], ins["mlp_b2"][p], s)
        kp = _prep_side_kan(ins["kan_ln_g0"][p], ins["kan_ln_b0"][p],
                            ins["kan_sl_b0"][p],
                            ins["kan_ln_g1"][p], ins["kan_ln_b1"][p],
                            ins["kan_sl_b1"][p], mixed[p], s)
        cst = np.concatenate([mp["b1"], mp["b2"], kp["bias0"], kp["bias1"],
                              kp["sc0"], kp["gb0"], kp["sc1"], kp["gb1"]],
                             axis=1)
        in_maps.append({
            "xm": xm_pair[p], "xk": xk_pair[p],
            "w1": mp["w1"], "w2": mp["w2"],
            "w0": kp["w0"], "w1k": kp["w1k"],
            "cst": np.ascontiguousarray(cst),
        })

    # ---- compile + run ----
    nc = _get_program()
    res = bass_utils.run_bass_kernel_spmd(nc, in_maps, core_ids=list(range(8)))
    last_run_info.clear()
    last_run_info.update(
        exec_time_ns=res.exec_time_ns,
        mean_exec_time_ns=res.mean_exec_time_ns,
        max_exec_time_core_id=res.max_exec_time_core_id,
        profile_json=res.profile_json,
        res=res,
    )

    # ---- host combine: out[t] = sum_e w[t,e] * (y_e_side0[t] + y_e_side1[t])
    out = np.zeros((T, H), np.float32)
    for p in range(E2):
        for (e, name) in ((p, "ym"), (E2 + p, "yk")):
            idx = shards[e][1]
            n = len(idx)
            if n:
                y = (res.results[2 * p][name].astype(np.float32)
                     + res.results[2 * p + 1][name].astype(np.float32))
                out[idx] += w_full[idx, e][:, None] * y[:, :n].T
    # overflow tokens (beyond capacity): exact host math
    for e, _idx, ovf in shards:
        if len(ovf):
            y = _host_expert(e, x[ovf], ins)
            out[ovf] += w_full[ovf, e][:, None] * y
    return out.reshape(hs.shape).astype(np.float32)

